# revision 9
# baseline (speedup 1.0000x reference)
"""Bilateral filter (7x7, dilation 1) Trainium2 Bass kernel — v2.

Problem: input [2, 18, 1024, 1024] f32.
  filterable f = input[:, :8]; params p = input[:, 8:]
  logw = -(sum_c p_c^2 (fn_c - f_c)^2 + px^2 dx^2 + py^2 dy^2)
  out[c] = sum_taps exp(logw) f_c(shifted) / sum_taps exp(logw), c < 3

Sharding: 8 cores x (one batch quarter: 256 rows). Host pre-interleaves
channels, converts to fp16, and pads a 3-px halo with sentinel 200.0 so
out-of-image taps get d^2 ~ 4e4 -> logw <= -40 -> weight 0 (fp16-safe:
d^2 <= 42k < 65504, no inf/NaN before the positive-only tree adds).

Per-core layout: rows on partitions (2 x 128), W in 4 chunks of 256 on the
free axis, 8 filterable channels interleaved.  Engine split per tap:
  DVE : d = sh - fc (fp16 2x), g = p2*d2 (2x), s1/s2 adds, t3 = w*fn3 (2x),
        quad-tree fp32 accumulation
  ACT : d2 = Square(d) in-place, w = Exp(-s2)
  Pool: tree rounds 1-2 of the 8->1 channel reduce
Center tap skipped entirely (w == 1 exactly): acc += fn3, wsum += 1.
"""

import sys

if "/opt/trn_rl_repo" not in sys.path:
    sys.path.insert(0, "/opt/trn_rl_repo")

import numpy as np

import concourse.bass as bass
import concourse.mybir as mybir
from concourse.bacc import Bacc
from concourse.tile import TileContext

FP32 = mybir.dt.float32
FP16 = mybir.dt.float16
AT = mybir.AluOpType

B, C_ALL, H, W = 2, 18, 1024, 1024
CF = 8                      # filterable channels
CO = 3                      # output channels
KS, RAD = 7, 3
HC = H * B // 8             # 256 output rows per core
HIN = HC + 2 * RAD          # 262 input rows per core (halo padded host-side)
WIN = W + 2 * RAD           # 1030 padded width
WC = 256                    # W chunk
NW = W // WC                # 4
NHB = HC // 128             # 2
SENT = 200.0                # sentinel padding (fp16-safe, forces weight ~0)
D2IDX = [3, 2, 1, 0, 1, 2, 3]              # index into [0,1,4,9]
D2VALS = [0.0, 1.0, 4.0, 9.0]

_CACHED = {}


def _ilv(ap, c=CF):
    """View flat [128, n*c] region as [128, n, c] (channel-interleaved)."""
    return ap.rearrange("p (x c) -> p x c", c=c)


def _pl(ap, c):
    """View flat [128, c*n] region as [128, c, n] (planar)."""
    return ap.rearrange("p (c x) -> p c x", c=c)


def build_nc():
    nc = Bacc()
    xf = nc.dram_tensor("xf", [HIN, WIN, CF], FP16, kind="ExternalInput")
    xp3 = nc.dram_tensor("xp3", [CO, HIN, WIN], FP16, kind="ExternalInput")
    xr = nc.dram_tensor("xr", [HC, W, CF], FP16, kind="ExternalInput")
    xs = nc.dram_tensor("xs", [2, HC, W], FP16, kind="ExternalInput")
    y = nc.dram_tensor("y", [CO, HC, W], FP32, kind="ExternalOutput")

    with TileContext(nc) as tc:
        with (
            tc.tile_pool(name="fpool", bufs=2) as fpool,
            tc.tile_pool(name="cpool", bufs=2) as cpool,
            tc.tile_pool(name="dpool", bufs=3) as dpool,
            tc.tile_pool(name="spool", bufs=4) as spool,
            tc.tile_pool(name="tpool", bufs=3) as tpool,
        ):
            for hb in range(NHB):
                for wck in range(NW):
                    _macro(nc, tc, xf, xp3, xr, xs, y,
                           fpool, cpool, dpool, spool, tpool, hb, wck)
    nc.compile()
    return nc


def _macro(nc, tc, xf, xp3, xr, xs, y, fpool, cpool, dpool, spool, tpool,
           hb, wck):
    w0 = wck * WC
    r0 = hb * 128
    wt = WC + 2 * RAD          # 262: tile col t <-> image col w0 - 3 + t

    # ---- load the 7 row-shifted filterable tile sets (pre-interleaved) ----
    F, P3 = [], []
    for oy in range(KS):
        Fi = fpool.tile([128, wt * CF], FP16, tag=f"F{oy}",
                        name=f"F{oy}_{hb}_{wck}")
        nc.sync.dma_start(out=Fi[:],
                          in_=xf[r0 + oy: r0 + oy + 128, w0: w0 + wt, :])
        F.append(Fi)
        Pi = fpool.tile([128, CO * wt], FP16, tag=f"P{oy}",
                        name=f"P{oy}_{hb}_{wck}")
        for c in range(CO):
            nc.sync.dma_start(
                out=Pi[:, c * wt: (c + 1) * wt],
                in_=xp3[c, r0 + oy: r0 + oy + 128, w0: w0 + wt])
        P3.append(Pi)
    Fc = _ilv(F[RAD][:, RAD * CF: (RAD + WC) * CF])       # center view
    Pc = _pl(P3[RAD][:], CO)[:, :, RAD: RAD + WC]         # center fn3 planar

    # ---- params: R2 = p^2 (interleaved), spatial combo table Asp ----
    praw = cpool.tile([128, WC * CF], FP16, tag="praw", name=f"pr_{hb}_{wck}")
    nc.sync.dma_start(out=praw[:], in_=xr[r0: r0 + 128, w0: w0 + WC, :])
    R2 = cpool.tile([128, WC * CF], FP16, tag="R2", name=f"R2_{hb}_{wck}")
    nc.scalar.activation(R2[:], praw[:], mybir.ActivationFunctionType.Square)

    sxy = cpool.tile([128, 2 * WC], FP16, tag="sxy", name=f"sxy_{hb}_{wck}")
    for k in range(2):
        nc.sync.dma_start(out=sxy[:, k * WC: (k + 1) * WC],
                          in_=xs[k, r0: r0 + 128, w0: w0 + WC])
    pq = cpool.tile([128, 2 * WC], FP16, tag="pq", name=f"pq_{hb}_{wck}")
    nc.scalar.activation(pq[:], sxy[:], mybir.ActivationFunctionType.Square)

    # Axy[0:4] = a*px2, Axy[4:8] = b*py2 for a,b in {0,1,4,9}  (TSP, 4x)
    Axy = cpool.tile([128, 8 * WC], FP16, tag="Axy", name=f"Axy_{hb}_{wck}")
    for ai, av in enumerate(D2VALS):
        nc.vector.tensor_scalar_mul(
            Axy[:, ai * WC: (ai + 1) * WC], pq[:, 0:WC], float(av))
        nc.vector.tensor_scalar_mul(
            Axy[:, (4 + ai) * WC: (5 + ai) * WC], pq[:, WC: 2 * WC], float(av))
    # Asp[a*4+b] = a*px2 + b*py2  >= 0  (w = exp(-(quad + Asp)))
    Asp = cpool.tile([128, 16 * WC], FP16, tag="Asp", name=f"Asp_{hb}_{wck}")
    for ai in range(4):
        for bi in range(4):
            k = ai * 4 + bi
            nc.vector.tensor_add(
                Asp[:, k * WC: (k + 1) * WC],
                Axy[:, ai * WC: (ai + 1) * WC],
                Axy[:, (4 + bi) * WC: (5 + bi) * WC])

    # ---- accumulators (fp16: mixed-dtype TT adds are ~7x slow on DVE) ----
    acc = cpool.tile([128, WC * CO], FP16, tag="acc", name=f"acc_{hb}_{wck}")
    wsum = cpool.tile([128, WC], FP16, tag="wsum", name=f"ws_{hb}_{wck}")
    # center tap contributes w == 1 exactly: acc = fn3, wsum = 1
    nc.gpsimd.memset(wsum[:], 1.0)
    nc.scalar.copy(_pl(acc[:], CO), Pc)

    taps = [(i, j) for i in range(KS) for j in range(KS) if not (i == RAD and j == RAD)]
    assert len(taps) == 48

    qt3 = []   # pending quad t3 tiles
    qw = []    # pending quad w tiles
    for ti, (i, j) in enumerate(taps):
        sh = _ilv(F[i][:, j * CF: (j + WC) * CF])
        d = dpool.tile([128, WC * CF], FP16, tag="d",
                       name=f"d_{hb}_{wck}_{i}_{j}")
        nc.vector.tensor_sub(_ilv(d[:]), sh, Fc)
        nc.scalar.activation(d[:], d[:], mybir.ActivationFunctionType.Square)
        nc.vector.tensor_mul(_ilv(d[:]), _ilv(R2[:]), _ilv(d[:]))
        # 8 -> 1 channel sum: single dense reduce (contiguous read + write)
        s2 = spool.tile([128, WC], FP16, tag="s2",
                        name=f"s2_{hb}_{wck}_{i}_{j}")
        with nc.allow_low_precision("fp16 logw sum, tolerance 2e-2"):
            nc.vector.tensor_reduce(s2[:], _ilv(d[:]),
                                    axis=mybir.AxisListType.X,
                                    op=AT.add)
        k = (D2IDX[j] * 4 + D2IDX[i]) * WC
        nc.vector.tensor_add(s2[:], s2[:], Asp[:, k: k + WC])
        w_t = spool.tile([128, WC], FP16, tag="w",
                         name=f"w_{hb}_{wck}_{i}_{j}")
        nc.scalar.activation(w_t[:], s2[:], mybir.ActivationFunctionType.Exp,
                             scale=-1.0)
        # t3 = w * fn3 (planar, fp16 2x)
        t3 = tpool.tile([128, CO * WC], FP16, tag="t3",
                        name=f"t3_{hb}_{wck}_{i}_{j}")
        w_b = w_t[:].unsqueeze(1).broadcast_to((128, CO, WC))
        nc.vector.tensor_mul(_pl(t3[:], CO), w_b,
                             _pl(P3[i][:], CO)[:, :, j: j + WC])
        qt3.append(t3)
        qw.append(w_t)
        if len(qt3) == 4:
            p01 = tpool.tile([128, CO * WC], FP16, tag="p01",
                             name=f"p01_{hb}_{wck}_{ti}")
            p23 = tpool.tile([128, CO * WC], FP16, tag="p23",
                             name=f"p23_{hb}_{wck}_{ti}")
            nc.gpsimd.tensor_add(p01[:], qt3[0][:], qt3[1][:])
            nc.gpsimd.tensor_add(p23[:], qt3[2][:], qt3[3][:])
            nc.vector.tensor_add(p01[:], p01[:], p23[:])
            nc.vector.tensor_add(acc[:], acc[:], p01[:])
            w01 = spool.tile([128, WC], FP16, tag="w01",
                             name=f"w01_{hb}_{wck}_{ti}")
            w23 = spool.tile([128, WC], FP16, tag="w23",
                             name=f"w23_{hb}_{wck}_{ti}")
            nc.gpsimd.tensor_add(w01[:], qw[0][:], qw[1][:])
            nc.gpsimd.tensor_add(w23[:], qw[2][:], qw[3][:])
            nc.vector.tensor_add(w01[:], w01[:], w23[:])
            nc.vector.tensor_add(wsum[:], wsum[:], w01[:])
            qt3, qw = [], []

    # ---- out = acc / wsum (convert fp16 accumulators to fp32 once) ----
    wsf = cpool.tile([128, WC], FP32, tag="wsf", name=f"wsf_{hb}_{wck}")
    nc.scalar.copy(wsf[:], wsum[:])
    accf = cpool.tile([128, WC * CO], FP32, tag="accf", name=f"af_{hb}_{wck}")
    nc.scalar.copy(accf[:], acc[:])
    rec = cpool.tile([128, WC], FP32, tag="rec", name=f"rec_{hb}_{wck}")
    nc.vector.reciprocal(rec[:], wsf[:])
    out3 = cpool.tile([128, WC * CO], FP32, tag="out3", name=f"o3_{hb}_{wck}")
    rec_b = rec[:].unsqueeze(1).broadcast_to((128, CO, WC))
    nc.vector.tensor_mul(_pl(out3[:], CO), rec_b, _pl(accf[:], CO))
    for c in range(CO):
        nc.sync.dma_start(out=y[c, r0: r0 + 128, w0: w0 + WC],
                          in_=out3[:, c * WC: (c + 1) * WC])


def shard_inputs(input):
    """input [2,18,1024,1024] f32 -> 8 per-core fp16 slabs."""
    input = np.asarray(input, dtype=np.float32)
    rows = H // 4
    in_maps = []
    for core in range(8):
        b, q = divmod(core, 4)
        r0 = q * rows
        fil = np.full((HIN, WIN, CF), SENT, dtype=np.float16)
        s_lo, s_hi = max(r0 - RAD, 0), min(r0 + rows + RAD, H)
        # [C,H,W] -> [H,W,C] interleaved
        fil[s_lo - (r0 - RAD): s_hi - (r0 - RAD), RAD: RAD + W, :] = (
            np.transpose(input[b, :CF, s_lo:s_hi, :], (1, 2, 0)))
        p3 = np.full((CO, HIN, WIN), SENT, dtype=np.float16)
        p3[:, s_lo - (r0 - RAD): s_hi - (r0 - RAD), RAD: RAD + W] = (
            input[b, :CO, s_lo:s_hi, :])
        pr = np.transpose(input[b, CF:2 * CF, r0: r0 + rows, :],
                          (1, 2, 0)).astype(np.float16)
        ps = input[b, 2 * CF: 2 * CF + 2, r0: r0 + rows, :].astype(np.float16)
        in_maps.append({"xf": np.ascontiguousarray(fil),
                        "xp3": np.ascontiguousarray(p3),
                        "xr": np.ascontiguousarray(pr),
                        "xs": np.ascontiguousarray(ps)})
    return in_maps


def assemble(results):
    out = np.empty((B, CO, H, W), dtype=np.float32)
    rows = H // 4
    for core in range(8):
        b, q = divmod(core, 4)
        out[b, :, q * rows: (q + 1) * rows, :] = results[core]["y"]
    return out


def kernel(input):
    from concourse.bass_utils import run_bass_kernel_spmd

    if "nc" not in _CACHED:
        _CACHED["nc"] = build_nc()
    in_maps = shard_inputs(input)
    res = run_bass_kernel_spmd(_CACHED["nc"], in_maps, list(range(8)))
    return assemble(res.results)
